# revision 1
# baseline (speedup 1.0000x reference)
"""Trainium2 Bass kernel for nn_Attention_6545530159375.

Full prefill attention (rope + GQA causal attention + output proj),
sharded over 8 NeuronCores as DP(batch=2) x TP(head-groups=4).

Per core (batch b, shard j): 8 q heads, 2 kv heads, full sequence.
  P1: q^T = (wq_j x_b^T) with rope          -> DRAM scratch [8,128,S]
  P2: k^T (rope) + v                        -> SBUF resident
  P3: per (qt, h): S^T tiles = k^T.T q^T, exp, (tri-mask), PV + ones-matmul
      denominator broadcast, reciprocal, normalize -> attnT SBUF resident
  P4: out = attnT.T woT^T, streamed to HBM.
All matmuls run as float32r (FP22) with fp32 PSUM accumulation.

Host side: transposes/permutes weights (rope pair-permutation baked into
wq/wk columns), runs the same NEFF on all 8 cores with per-core inputs,
then sums the 4 TP partials per batch.
"""

import sys

if "/opt/trn_rl_repo" not in sys.path:
    sys.path.insert(0, "/opt/trn_rl_repo")

import numpy as np

B, S, D, H, KV, HD = 2, 2048, 4096, 32, 8, 128
TPG = 4                 # tensor-parallel groups (x2 data-parallel = 8 cores)
HL = H // TPG           # 8 q heads per core
KVL = KV // TPG         # 2 kv heads per core
FL = HL * HD            # 1024 local features
NREP = HL // KVL * KVL // KVL  # unused; q head h -> kv head h // (HL // KVL)
QT = 512                # query tile (moving dim)
NQT = S // QT           # 4
NKT = S // 128          # 16 k-chunks
ND = D // QT            # 8 output d-chunks
SCALE = 1.0 / float(np.sqrt(HD))
EXP_BIAS = -2.0         # constant shift inside exp; cancels in softmax

_cache = {}


def _build(causal: bool):
    import concourse.mybir as mybir
    import concourse.tile as tile
    from concourse import bacc

    dt = mybir.dt
    f32 = dt.float32
    f16 = dt.float16
    AF = mybir.ActivationFunctionType
    ALU = mybir.AluOpType

    nc = bacc.Bacc()
    xT = nc.dram_tensor("xT", [D, S], f16, kind="ExternalInput")
    wqT = nc.dram_tensor("wqT", [D, FL], f16, kind="ExternalInput")
    wkT = nc.dram_tensor("wkT", [D, KVL * HD], f16, kind="ExternalInput")
    wvT = nc.dram_tensor("wvT", [D, KVL * HD], f16, kind="ExternalInput")
    woT = nc.dram_tensor("woT", [FL, D], f16, kind="ExternalInput")
    onesC = nc.dram_tensor("onesC", [128, 128], f16, kind="ExternalInput")
    zeroC = nc.dram_tensor("zeroC", [128, 384], f16, kind="ExternalInput")
    cosP = nc.dram_tensor("cosP", [128, S], f32, kind="ExternalInput")
    sinP = nc.dram_tensor("sinP", [128, S], f32, kind="ExternalInput")
    if causal:
        tri = nc.dram_tensor("tri", [4, 128, QT], f16, kind="ExternalInput")
    else:
        maskT = nc.dram_tensor("maskT", [S, S], f16, kind="ExternalInput")
    outp = nc.dram_tensor("outp", [S, D], f32, kind="ExternalOutput")

    NDCH = D // 128  # 32 contraction chunks

    with tile.TileContext(nc) as tc:
        with (
            tc.tile_pool(name="const", bufs=1) as constp,
            tc.tile_pool(name="dram", bufs=1, space="DRAM") as dramp,
        ):
            ones = constp.tile([128, 128], f16)
            nc.gpsimd.dma_start(ones, onesC[:, :])
            zeros_sb = constp.tile([128, 384], f16)
            nc.gpsimd.dma_start(zeros_sb, zeroC[:, :])
            biasT = constp.tile([128, 1], f32)
            nc.vector.memset(biasT, EXP_BIAS)
            qT_dram = dramp.tile([HL, 128, S], f32)

            def rope(dst, raw, swp, qt, pool, cos_sb, sin_sb):
                """raw=[r|i] rows, swp=[i|r] rows (pre-swapped via DMA).
                sin_sb rows 0:64 hold -sin, rows 64:128 hold +sin, so
                dst = raw*cos + swp*sin_signed gives the rope rotation."""
                c = cos_sb[:, qt * QT:(qt + 1) * QT]
                s = sin_sb[:, qt * QT:(qt + 1) * QT]
                tmp = pool.tile([128, QT], f32, name="ropetmp", tag="ropetmp")
                nc.vector.tensor_tensor(dst, raw, c, ALU.mult)
                nc.vector.tensor_tensor(tmp, swp, s, ALU.mult)
                nc.vector.tensor_tensor(dst, dst, tmp, ALU.add)

            # trig/kv pools open early so their loads overlap P1 compute
            ctx_trig = tc.tile_pool(name="trig", bufs=1)
            trigp = ctx_trig.__enter__()
            ctx_kv = tc.tile_pool(name="kv", bufs=1)
            kvp = ctx_kv.__enter__()
            cos_sb = trigp.tile([128, S], f32)
            sin_sb = trigp.tile([128, S], f32)
            nc.gpsimd.dma_start(cos_sb, cosP[:, :])
            nc.gpsimd.dma_start(sin_sb, sinP[:, :])
            if causal:
                tri_sb = trigp.tile([128, 4, QT], f16)
                for p in range(4):
                    nc.gpsimd.dma_start(tri_sb[:, p, :], tri[p])
            kT_sb = [kvp.tile([128, S], f16, name=f"kT{i}")
                     for i in range(KVL)]
            v_sb = [kvp.tile([128, NKT, 128], f16, name=f"v{i}")
                    for i in range(KVL)]

            # k/v weights preload during P1 (pool opened early so the
            # DMAs have no SBUF address dependency on P1's wq pool)
            ctx_wkv = tc.tile_pool(name="p2wkv", bufs=1)
            wkvp = ctx_wkv.__enter__()
            wk_res = wkvp.tile([128, NDCH, KVL * HD], f16, name="wkr")
            wv_res = wkvp.tile([128, NDCH, KVL * HD], f16, name="wvr")
            for d in range(NDCH):
                nc.gpsimd.dma_start(
                    wk_res[:, d, :], wkT[d * 128:(d + 1) * 128, :])
                nc.gpsimd.dma_start(
                    wv_res[:, d, :], wvT[d * 128:(d + 1) * 128, :])

            # -------- P1: q projection -> DRAM (unroped) --------
            # psum split 6+2: P2's pool reuses banks 0-5 as soon as heads
            # 0-5 drain, overlapping with heads 6-7 finishing.
            ctx_pp_b = tc.tile_pool(name="p1psumB", bufs=1, space="PSUM")
            pp_b = ctx_pp_b.__enter__()
            with (
                tc.tile_pool(name="p1wq", bufs=1) as wqp,
                tc.tile_pool(name="p1psum", bufs=1, space="PSUM") as pp,
                tc.tile_pool(name="p1x", bufs=8) as xp,
                tc.tile_pool(name="p1qs", bufs=6) as qsp,
            ):
                wq_res = wqp.tile([128, NDCH, FL], f16)
                for qt in range(NQT):
                    qpsum = [(pp if h < 6 else pp_b).tile(
                        [128, QT], f32, name=f"qp{h}", tag=f"qp{h}")
                             for h in range(HL)]
                    for d in range(NDCH):
                        if qt == 0:
                            nc.sync.dma_start(
                                wq_res[:, d, :], wqT[d * 128:(d + 1) * 128, :])
                        xt = xp.tile([128, QT], f16, tag="x")
                        nc.sync.dma_start(
                            xt, xT[d * 128:(d + 1) * 128, qt * QT:(qt + 1) * QT])
                        for h in range(HL):
                            nc.tensor.matmul(
                                qpsum[h], wq_res[:, d, h * 128:(h + 1) * 128],
                                xt, start=(d == 0), stop=(d == NDCH - 1))
                    for h in range(HL):
                        qs = qsp.tile([128, QT], f32, tag="qrope")
                        nc.scalar.copy(qs, qpsum[h])
                        nc.sync.dma_start(
                            qT_dram[h, :, qt * QT:(qt + 1) * QT], qs)

            with (
                tc.tile_pool(name="qrope", bufs=1) as qrp,
                tc.tile_pool(name="qtmp", bufs=6) as qtp,
            ):
                qrope_sb = [None] * (NQT * HL)

                def prerope_qt(qt):
                    for h in range(HL):
                        qraw = qtp.tile([128, QT], f32, tag="qraw")
                        nc.gpsimd.dma_start(
                            qraw, qT_dram[h, :, qt * QT:(qt + 1) * QT])
                        qswp = qtp.tile([128, QT], f32, tag="qswp")
                        nc.gpsimd.dma_start(
                            qswp[0:64],
                            qT_dram[h, 64:128, qt * QT:(qt + 1) * QT])
                        nc.gpsimd.dma_start(
                            qswp[64:128],
                            qT_dram[h, 0:64, qt * QT:(qt + 1) * QT])
                        qtile = qrp.tile([128, QT], f16, name=f"qr{qt}_{h}")
                        rope(qtile, qraw, qswp, qt, qtp, cos_sb, sin_sb)
                        qrope_sb[qt * HL + h] = qtile

                # -------- P2: k (rope) + v -> SBUF --------
                with (
                    tc.tile_pool(name="p2psum", bufs=1, space="PSUM") as pp2,
                    tc.tile_pool(name="p2x", bufs=8) as xp2,
                    tc.tile_pool(name="p2rope", bufs=4) as ropep,
                ):
                    for qt in range(NQT):
                        kpsum = [pp2.tile([128, QT], f32, name=f"kp{i}",
                                          tag=f"kp{i}") for i in range(KVL)]
                        vpsum = [pp2.tile([128, KVL * HD], f32, name=f"vp{i}",
                                          tag=f"vp{i}") for i in range(4)]
                        for d in range(NDCH):
                            xt = xp2.tile([128, QT], f16, tag="x2")
                            nc.sync.dma_start(
                                xt, xT[d * 128:(d + 1) * 128,
                                       qt * QT:(qt + 1) * QT])
                            for i in range(KVL):
                                nc.tensor.matmul(
                                    kpsum[i],
                                    wk_res[:, d, i * 128:(i + 1) * 128], xt,
                                    start=(d == 0), stop=(d == NDCH - 1))
                            for t4 in range(4):
                                nc.tensor.matmul(
                                    vpsum[t4], xt[:, t4 * 128:(t4 + 1) * 128],
                                    wv_res[:, d, :],
                                    start=(d == 0), stop=(d == NDCH - 1))
                        for i in range(KVL):
                            ktmp = ropep.tile([128, QT], f32, tag="ktmp")
                            nc.scalar.copy(ktmp, kpsum[i])
                            kswp = ropep.tile([128, QT], f32, tag="kswp")
                            nc.sync.dma_start(kswp[0:64], ktmp[64:128])
                            nc.sync.dma_start(kswp[64:128], ktmp[0:64])
                            rope(kT_sb[i][:, qt * QT:(qt + 1) * QT],
                                 ktmp, kswp, qt, ropep, cos_sb, sin_sb)
                            for t4 in range(4):
                                nc.scalar.copy(
                                    v_sb[i][:, qt * 4 + t4, :],
                                    vpsum[t4][:, i * 128:(i + 1) * 128])
                        prerope_qt(qt)
                ctx_pp_b.__exit__(None, None, None)

                with tc.tile_pool(name="attn", bufs=1) as attnp:
                    attnT_sb = [attnp.tile([128, S], f16, name=f"aT{h}")
                                for h in range(HL)]

                    # -------- P3: attention --------
                    with (
                        tc.tile_pool(name="p3sp", bufs=4, space="PSUM") as spp,
                        tc.tile_pool(name="p3o", bufs=3, space="PSUM") as opp,
                        tc.tile_pool(name="p3d", bufs=1, space="PSUM") as dpp,
                        tc.tile_pool(name="p3pt", bufs=8 if causal else 5) as ptp,
                        tc.tile_pool(name="p3acc", bufs=3 if causal else 2) as accp,
                        tc.tile_pool(name="p3rec", bufs=3) as recp,
                        tc.tile_pool(name="p3m", bufs=1 if causal else NKT + 1) as mp3,
                    ):
                        for qt in range(NQT):
                            if not causal:
                                mtiles = []
                                for kt in range(NKT):
                                    mt = mp3.tile([128, QT], f16, name="mt",
                                                  tag="mt")
                                    nc.sync.dma_start(
                                        mt, maskT[kt * 128:(kt + 1) * 128,
                                                  qt * QT:(qt + 1) * QT])
                                    mtiles.append(mt)
                            nkt = 4 * (qt + 1) if causal else NKT
                            for h in range(HL):
                                kvh = h // (HL // KVL)
                                qtile = qrope_sb[qt * HL + h]
                                opsum = opp.tile([128, QT], f32, tag="o")
                                ptacc = accp.tile([128, QT], f16, tag="pa")
                                for kt in range(nkt):
                                    p = kt - qt * 4 if causal else -1
                                    z = max(p, 0) * 128  # masked col prefix
                                    sp = spp.tile([128, QT], f32, tag="s")
                                    nc.tensor.matmul(
                                        sp[:, z:],
                                        kT_sb[kvh][:, kt * 128:(kt + 1) * 128],
                                        qtile[:, z:], start=True, stop=True)
                                    if not causal:
                                        nc.vector.tensor_tensor(
                                            sp, sp, mtiles[kt], ALU.add)
                                    pt = ptp.tile([128, QT], f16, tag="pt")
                                    if z:
                                        nc.vector.tensor_copy(
                                            pt[:, :z], zeros_sb[:, :z])
                                    nc.scalar.activation(
                                        pt[:, z:], sp[:, z:], AF.Exp,
                                        bias=biasT, scale=SCALE)
                                    if causal and p >= 0:
                                        nc.vector.tensor_tensor(
                                            pt[:, z:z + 128], pt[:, z:z + 128],
                                            tri_sb[:, 0, 0:128], ALU.mult)
                                    nc.tensor.matmul(
                                        opsum, v_sb[kvh][:, kt, :], pt,
                                        start=(kt == 0), stop=(kt == nkt - 1))
                                    if kt == 0:
                                        nc.vector.tensor_copy(ptacc, pt)
                                    else:
                                        nc.vector.tensor_tensor(
                                            ptacc, ptacc, pt, ALU.add)
                                dpsum = dpp.tile([128, QT], f32, tag="d")
                                nc.tensor.matmul(
                                    dpsum, ones, ptacc, start=True, stop=True)
                                rec = recp.tile([128, QT], f32, tag="rec")
                                nc.vector.reciprocal(rec, dpsum)
                                nc.vector.tensor_tensor(
                                    attnT_sb[h][:, qt * QT:(qt + 1) * QT],
                                    opsum, rec, ALU.mult)

                    # -------- P4: output projection --------
                    with (
                        tc.tile_pool(name="p4psum", bufs=4, space="PSUM") as pp4,
                        tc.tile_pool(name="p4w", bufs=3) as wp4,
                        tc.tile_pool(name="p4o", bufs=6) as op4,
                    ):
                        for dd in range(ND):
                            wot = wp4.tile([128, HL, QT], f16, tag="wo")
                            nc.sync.dma_start(
                                wot,
                                woT[:, dd * QT:(dd + 1) * QT].rearrange(
                                    "(fo p) n -> p fo n", p=128))
                            for tcn in range(S // 128):
                                wpsum = pp4.tile([128, QT], f32, tag="wps")
                                for f in range(HL):
                                    nc.tensor.matmul(
                                        wpsum,
                                        attnT_sb[f][:, tcn * 128:(tcn + 1) * 128],
                                        wot[:, f, :],
                                        start=(f == 0), stop=(f == HL - 1))
                                osb = op4.tile([128, QT], f32, tag="osb")
                                nc.vector.tensor_copy(osb, wpsum)
                                nc.sync.dma_start(
                                    outp[tcn * 128:(tcn + 1) * 128,
                                         dd * QT:(dd + 1) * QT], osb)
            ctx_wkv.__exit__(None, None, None)
            ctx_kv.__exit__(None, None, None)
            ctx_trig.__exit__(None, None, None)
    nc.finalize()
    return nc


_PERM = np.concatenate([np.arange(0, HD, 2), np.arange(1, HD, 2)])


def _is_causal(mask):
    if mask.shape != (S, S):
        return False
    tril = np.tril(np.ones((S, S), dtype=bool))
    if not np.all(mask[tril] == 0.0):
        return False
    return bool(np.all(mask[~tril] <= -1e8))


def kernel(x, wq, wk, wv, wo, cos, sin, mask, start_pos):
    from concourse import bass_utils

    x = np.asarray(x, np.float32)
    wq = np.asarray(wq, np.float32)
    wk = np.asarray(wk, np.float32)
    wv = np.asarray(wv, np.float32)
    wo = np.asarray(wo, np.float32)
    cos = np.asarray(cos, np.float32)
    sin = np.asarray(sin, np.float32)
    mask = np.asarray(mask, np.float32)

    causal = _is_causal(mask)
    key = causal
    if key not in _cache:
        _cache[key] = _build(causal)
    nc = _cache[key]

    ones_c = np.ones((128, 128), np.float16)
    zero_c = np.zeros((128, 384), np.float16)
    cosP = np.ascontiguousarray(np.tile(cos.T, (2, 1)))
    sinP = np.ascontiguousarray(np.concatenate([-sin.T, sin.T], axis=0))
    if causal:
        k_idx = np.arange(128)[:, None]
        q_idx = np.arange(QT)[None, :]
        tri = np.stack(
            [(p * 128 + k_idx <= q_idx).astype(np.float16) for p in range(4)])
    else:
        # fp16 additive mask, pre-scaled by sqrt(HD); clamp so -1e9
        # sentinels become a finite, safely-saturating fp16 value
        maskT = np.clip(mask.T * np.float64(np.sqrt(HD)),
                        -60000.0, 60000.0).astype(np.float16)
        maskT = np.ascontiguousarray(maskT)

    in_maps = []
    shard_data = []
    for j in range(TPG):
        wq_j = wq[j * FL:(j + 1) * FL].reshape(HL, HD, D)[:, _PERM, :]
        wqT = np.ascontiguousarray(wq_j.reshape(FL, D).T, np.float16)
        wk_j = wk[j * KVL * HD:(j + 1) * KVL * HD].reshape(KVL, HD, D)[:, _PERM, :]
        wkT = np.ascontiguousarray(wk_j.reshape(KVL * HD, D).T, np.float16)
        wvT = np.ascontiguousarray(
            wv[j * KVL * HD:(j + 1) * KVL * HD].T, np.float16)
        woT = np.ascontiguousarray(wo[:, j * FL:(j + 1) * FL].T, np.float16)
        shard_data.append((wqT, wkT, wvT, woT))

    xTs = [np.ascontiguousarray(x[b].T, np.float16) for b in range(B)]
    for c in range(8):
        b, j = divmod(c, TPG)
        wqT, wkT, wvT, woT = shard_data[j]
        m = {
            "xT": xTs[b], "wqT": wqT, "wkT": wkT, "wvT": wvT, "woT": woT,
            "cosP": cosP, "sinP": sinP, "onesC": ones_c, "zeroC": zero_c,
        }
        if causal:
            m["tri"] = tri
        else:
            m["maskT"] = maskT
        in_maps.append(m)

    global _last_in_maps
    _last_in_maps = in_maps
    res = bass_utils.run_bass_kernel_spmd(nc, in_maps, core_ids=list(range(8)))
    out = np.zeros((B, S, D), np.float32)
    for c in range(8):
        b = c // TPG
        out[b] += res.results[c]["outp"]
    return out

